# revision 42
# baseline (speedup 1.0000x reference)
"""CrossPSDLoss Trainium2 kernel (fp8 DoubleRow direct-DFT version).

Math (from the reference):
  res = target - pred; both [1024, 16384] f32.
  cross rows i=0..15: row i = concat_b x[b, 1024*i : 1024*(i+1)]  (length 1048576)
  Welch per row: 511 frames of 4096 (stride 2048), periodic-hann*2 window,
  rFFT, power, sum over frames -> S[k].  Loss only uses rows 8..15 and
  frequency bins 21..499, and the /T factors cancel in the ratio:
     out = (2/480) * sum_{row=8..15} sum_{kb=21..499} S_res[row,kb]/S_tgt[row,kb]

Sharding: one Welch row per NeuronCore (8 rows, 8 cores); each core consumes
only its [1024, 1024] column slice of pred/target.  No collectives; the host
sums the 8 per-core partial scalars.

Per-core pipeline (vs the bf16 folded baseline; each step validated):
  - fp8(e4m3) everywhere on the DFT path; MatmulPerfMode.DoubleRow packs two
    128-deep k-tiles per matmul at 0.5 cycles/output-column -> 4x the bf16
    GEMM rate.  Walrus requires the stationary free size (2M) to be a
    multiple of 64 -> bin chunks of 128/96, not 120.  End-to-end rel err
    ~1.2e-4 (tol 2e-2).
  - NO even/odd fold (its U/V builds cost more DVE time than the PE time
    they save at fp8 rates): direct 32-k-tile contraction, 16 DoubleRow
    matmuls per (input, trig, bin-chunk) PSUM group.
  - Non-redundant x layout: frames overlap 50%, so sample (j,g) of the
    [128 part, 16 slot, 512 grp] buffer serves k-tile j at frame g AND
    k-tile j+16 at frame g-1 via a shifted AP.  Halves x DMA.
  - res transform by linearity in PSUM: DFT(res) = DFT(tgt) - DFT(pred),
    evaluated as (copy of tgt PSUM in SBUF) - (pred PSUM) on DVE (the
    engine can read at most one PSUM operand per instruction).
  - Every SBUF tile is written by exactly ONE DMA (x and weight tensors
    split into lo/hi tiles): the tile dependency tracker otherwise hangs
    first-use matmuls on later unrelated DMAs.
  - All 8 tgt GEMM groups run first (their weights stream during them),
    pred groups after, with xsp DMA'd last: minimizes PE stalls.
  - PE p-state warmup: dummy matmuls during the DMA lead-in so the real
    GEMMs run at 2.4 GHz from the first instruction.
  - Host prep is layout/dtype only (slice, reshape, transpose, fp8 cast).
"""

import os
import sys
from contextlib import ExitStack

import numpy as np
import ml_dtypes

for _p in ("/opt/trn_rl_repo", "/root/.axon_site/_ro/trn_rl_repo"):
    if os.path.isdir(_p) and _p not in sys.path:
        sys.path.insert(0, _p)

import concourse.bass as bass
import concourse.mybir as mybir
from concourse import bacc, tile
from concourse.bass_utils import run_bass_kernel_spmd

FP8 = ml_dtypes.float8_e4m3

NPERSEG = 4096
NSEG = 511
NBINS = 479          # bins 21..499
# Dual-fp8 ldweights requires the stationary free size (2M) to be a
# multiple of 64, so bin-chunks must be multiples of 32: pad 479 bins to
# 480 = 128*3 + 96 with one all-zero weight column; RCHUNKS excludes the
# pad bin from the ratio stage (its Et is 0 -> 1/0 would NaN the sum).
CHUNKS = [128, 128, 128, 96]
RCHUNKS = [128, 128, 128, 95]
N_CORES = 8
ROW0 = 8             # first Welch row that matters

# x-buffer slot s (0..15) holds k-tile s; k-tiles 16..31 alias slots 0..15
# shifted one frame.  Weight slot s holds k-tile PERM[s]: the first 16
# weight slots pair with x slots 0..7 (both plain and shifted), so the
# lo half of each GEMM group touches only lo-half tiles.
PERM = list(range(0, 8)) + list(range(16, 24)) + list(range(8, 16)) + list(range(24, 32))
_SLOT_OF = {t: s for s, t in enumerate(PERM)}
# matmul issue order: by x-buffer quarter (slots 0-3, 4-7, 8-11, 12-15),
# so each 4-matmul block only needs one quarter of the x DMA stream.
# Blocks 0-1 read the lo weight tile, blocks 2-3 the hi tile.
KK_ORDER = [0, 1, 8, 9, 2, 3, 10, 11, 4, 5, 12, 13, 6, 7, 14, 15]

N_WARMUP = int(os.environ.get("KERNEL_N_WARMUP", "20"))


def _build_nc() -> bass.Bass:
    # Bacc (not bass.Bass): its compile() runs generate_event_semaphores(),
    # which splits multi-semaphore waits into event-sem chains — TRN2
    # instructions support at most one wait each.
    nc = bacc.Bacc("TRN2", target_bir_lowering=False, debug=False,
                   num_devices=N_CORES)
    dt = mybir.dt
    DR = mybir.MatmulPerfMode.DoubleRow

    xst_d = nc.dram_tensor("xst", [128, 16, 512], dt.float8e4,
                           kind="ExternalInput")
    xsp_d = nc.dram_tensor("xsp", [128, 16, 512], dt.float8e4,
                           kind="ExternalInput")
    wd = {}
    for trig in ("c", "s"):
        for ci, rows in enumerate(CHUNKS):
            nm = f"w{trig}{ci}"
            wd[nm] = nc.dram_tensor(nm, [128, 32, rows], dt.float8e4,
                                    kind="ExternalInput")
    out_d = nc.dram_tensor("out", [1, 1], dt.float32, kind="ExternalOutput")

    with ExitStack() as ctx:
        tc = ctx.enter_context(tile.TileContext(nc))
        xpool = ctx.enter_context(tc.tile_pool(name="x", bufs=1))
        wpool = ctx.enter_context(tc.tile_pool(name="w", bufs=1))
        pst_pool = ctx.enter_context(tc.tile_pool(name="pst", bufs=3, space="PSUM"))
        psp_pool = ctx.enter_context(tc.tile_pool(name="psp", bufs=2, space="PSUM"))
        ps1 = ctx.enter_context(tc.tile_pool(name="ps1", bufs=1, space="PSUM"))
        psb = ctx.enter_context(tc.tile_pool(name="psb", bufs=1, space="PSUM"))
        ptpool = ctx.enter_context(tc.tile_pool(name="pt", bufs=1))
        dpool = ctx.enter_context(tc.tile_pool(name="d", bufs=2))
        scpool = ctx.enter_context(tc.tile_pool(name="sc", bufs=3))
        stat = ctx.enter_context(tc.tile_pool(name="stat", bufs=1))

        # PE p-state warmup (see module docstring).  256-wide so the engine
        # time per matmul (213 ns at the mid p-state) exceeds the PE.SEQ
        # dispatch cost (~142 ns for ldweights+matmult) — narrower warmups
        # throttle on the sequencer and delay the first real GEMM dispatch.
        if N_WARMUP:
            wa = stat.tile([1, 256], dt.bfloat16)
            nc.vector.memset(wa[:, :], 1.0)
            wps = ps1.tile([128, 256], dt.float32)
            for _ in range(N_WARMUP):
                nc.tensor.matmul(wps[:, :], wa[:1, :128], wa[:1, :],
                                 start=True, stop=True)
            warm_junk = stat.tile([1, 1], dt.float32)
            nc.vector.tensor_copy(warm_junk[:1, :1], wps[0:1, 0:1])

        # x quarter tiles: one DMA per tile (finer grain than the weights so
        # the first GEMM blocks start as soon as their slice lands; weight
        # quarters would sink below the 500ns DMA descriptor floor).
        xs = {}
        for nm in ("t", "p"):
            for q in range(4):
                xs[(nm, q)] = xpool.tile([128, 4, 512], dt.float8e4,
                                         tag=f"xs{nm}q{q}", name=f"xs{nm}q{q}")
        wsb = {}
        for trig in ("c", "s"):
            for ci, rows in enumerate(CHUNKS):
                for half in (0, 1):
                    nm = f"w{trig}{ci}{'lo' if half == 0 else 'hi'}"
                    wsb[(trig, ci, half)] = wpool.tile(
                        [128, 16, rows], dt.float8e4, tag=nm, name=nm)

        def dma_w(trig, ci):
            rows = CHUNKS[ci]
            for half in (0, 1):
                nc.sync.dma_start(wsb[(trig, ci, half)][:, :, :],
                                  wd[f"w{trig}{ci}"][:, 16 * half:16 * half + 16, :])

        def dma_x(nm, dram, q):
            nc.sync.dma_start(xs[(nm, q)][:, :, :],
                              dram[:, 4 * q:4 * q + 4, :])

        # DMA order = PE need order.  Chunk 3 (96 bins, 1.18us vs 1.48us of
        # DMA) goes first everywhere: the first GEMM group is gated by
        # init + its weight chunk + xst, so the small chunk shaves the
        # critical prefix; the later weight stream still stays ahead of
        # the PE's 1.7us/group consumption.
        nc.sync.dma_start(wsb[("c", 3, 0)][:, :, :], wd["wc3"][:, 0:16, :])
        dma_x("t", xst_d, 0)
        nc.sync.dma_start(wsb[("c", 3, 1)][:, :, :], wd["wc3"][:, 16:32, :])
        for q in (1, 2, 3):
            dma_x("t", xst_d, q)
        dma_w("c", 0)
        dma_w("c", 1)
        dma_w("c", 2)
        dma_w("s", 3)
        dma_w("s", 0)
        dma_w("s", 1)
        dma_w("s", 2)
        for q in range(4):
            dma_x("p", xsp_d, q)

        def gemm(ps_t, inp, trig, ci, f_lo=0, f_hi=NSEG, out0=0):
            """One PSD transform over frames [f_lo, f_hi) into psum columns
            [out0, out0 + f_hi - f_lo): 16 DoubleRow matmuls (32 k-tiles of
            128), lo-half tiles first."""
            rows = CHUNKS[ci]
            n = f_hi - f_lo
            for pos, kk in enumerate(KK_ORDER):
                t0 = 2 * kk
                s0 = _SLOT_OF[t0]
                sx = t0 % 16
                xtile = xs[(inp, sx // 4)]
                if t0 < 16:
                    rhs = xtile[:, (sx % 4):(sx % 4) + 2, f_lo:f_hi]
                else:
                    rhs = xtile[:, (sx % 4):(sx % 4) + 2, f_lo + 1:f_hi + 1]
                wtile = wsb[(trig, ci, 0 if s0 < 16 else 1)]
                nc.tensor.matmul(
                    ps_t[:rows, out0:out0 + n],
                    wtile[:, s0 % 16:s0 % 16 + 2, :rows],
                    rhs,
                    start=(pos == 0),
                    stop=(pos == 15),
                    perf_mode=DR,
                )

        CORDER = [3, 0, 1, 2]
        GROUPS = [(t, c) for t in ("c", "s") for c in CORDER]

        # Phase 1: all 8 tgt transforms; each PSUM is copied to SBUF (pt_sb)
        # then squared+accumulated (Et) off the PE critical path.
        PT = {}
        E = {}
        for trig, ci in GROUPS:
            rows = CHUNKS[ci]
            ps_t = pst_pool.tile([128, NSEG], dt.float32, tag="ps_t")
            gemm(ps_t, "t", trig, ci)
            pt_sb = ptpool.tile([128, NSEG], dt.float32,
                                tag=f"pt_{trig}{ci}", name=f"pt_{trig}{ci}")
            nc.vector.tensor_copy(pt_sb[:rows, :], ps_t[:rows, :])
            PT[(trig, ci)] = pt_sb
            et = stat.tile([128, 1], dt.float32, tag=f"Et_{trig}{ci}",
                           name=f"Et_{trig}{ci}")
            tmp_t = scpool.tile([128, NSEG], dt.float32, tag="sq_t")
            nc.scalar.activation(
                out=tmp_t[:rows, :], in_=pt_sb[:rows, :],
                func=mybir.ActivationFunctionType.Square,
                accum_out=et[:rows, :])
            E[(1, trig, ci)] = et

        # Denominator sums + scaled reciprocals (2/480 folded in here, all
        # off the critical path once phase 1 is done).
        REC = {}
        for ci, rows in enumerate(RCHUNKS):
            st_ = stat.tile([128, 1], dt.float32, tag=f"ST{ci}",
                            name=f"ST{ci}")
            rec = stat.tile([128, 1], dt.float32, tag=f"REC{ci}",
                            name=f"REC{ci}")
            rec2 = stat.tile([128, 1], dt.float32, tag=f"REC2{ci}",
                             name=f"REC2{ci}")
            nc.vector.tensor_add(st_[:rows, :], E[(1, "c", ci)][:rows, :],
                                 E[(1, "s", ci)][:rows, :])
            nc.vector.reciprocal(rec[:rows, :], st_[:rows, :])
            nc.vector.tensor_scalar_mul(rec2[:rows, :], rec[:rows, :],
                                        2.0 / 480.0)
            REC[ci] = rec2

        # Phase 2: pred transforms; d = pt_sb - ps_p (one PSUM operand),
        # Er = sum_f d^2 via ACT Square+accum.  The very last group is
        # frame-split (448 + 63) into two PSUM groups on separate banks so
        # most of its sub runs while the PE finishes the 63-frame remainder
        # (shortens the tail-critical chain).
        FCUTS = [0, 384, 480, NSEG]
        for gi, (trig, ci) in enumerate(GROUPS):
            rows = CHUNKS[ci]
            last = (gi == len(GROUPS) - 1)
            ps_p = psp_pool.tile([128, NSEG], dt.float32, tag="ps_p")
            pt_sb = PT[(trig, ci)]
            d = dpool.tile([128, NSEG], dt.float32, tag="d")
            if last:
                # pieces land in separate PSUM banks (a matmul group start
                # zeroes its whole 2KB bank); the tgt pool is idle by now.
                ps_pb = psb.tile([128, FCUTS[2] - FCUTS[1]], dt.float32,
                                 tag="ps_pb", name="ps_pb")
                # same tag/shape as the (long-idle) tgt psum tiles so the
                # pst pool keeps its 1-bank buf size
                ps_pc = pst_pool.tile([128, NSEG], dt.float32,
                                      tag="ps_t", name="ps_pc")
                pieces = [ps_p, ps_pb, ps_pc]
                for (f0, f1), pp in zip(zip(FCUTS, FCUTS[1:]), pieces):
                    gemm(pp, "p", trig, ci, f0, f1, out0=0)
                for (f0, f1), pp in zip(zip(FCUTS, FCUTS[1:]), pieces):
                    nc.vector.tensor_sub(d[:rows, f0:f1],
                                         pt_sb[:rows, f0:f1],
                                         pp[:rows, :f1 - f0])
            else:
                gemm(ps_p, "p", trig, ci)
                nc.vector.tensor_sub(d[:rows, :], pt_sb[:rows, :],
                                     ps_p[:rows, :])
            er = stat.tile([128, 1], dt.float32, tag=f"Er_{trig}{ci}",
                           name=f"Er_{trig}{ci}")
            tmp_r = scpool.tile([128, NSEG], dt.float32, tag="sq_r")
            nc.scalar.activation(
                out=tmp_r[:rows, :], in_=d[:rows, :],
                func=mybir.ActivationFunctionType.Square,
                accum_out=er[:rows, :])
            E[(0, trig, ci)] = er

        # Finale: sum_bins (Er_c+Er_s) * rec as PE dot products accumulated
        # into one PSUM scalar (partition-dim reduce for free).
        tot = ps1.tile([1, 1], dt.float32)
        for pos, ci in enumerate(CORDER):
            rows = RCHUNKS[ci]
            sr = stat.tile([128, 1], dt.float32, tag=f"SR{ci}", name=f"SR{ci}")
            nc.vector.tensor_add(sr[:rows, :], E[(0, "c", ci)][:rows, :],
                                 E[(0, "s", ci)][:rows, :])
            nc.tensor.matmul(tot[:1, :1], sr[:rows, :1], REC[ci][:rows, :1],
                             start=(pos == 0), stop=(pos == 3))
        red = stat.tile([1, 1], dt.float32)
        nc.vector.tensor_copy(red[:1, :1], tot[:1, :1])
        nc.sync.dma_start(out_d[:, :], red[:1, :1])

    nc.compile()
    return nc


def _build_w():
    """fp8 DFT weights in the [part, slot, bin-chunk] layout:
      w{c,s}{ci}[p, s, n] = win[k] * {cos,sin}(2 pi k (21 + n0 + n) / 4096),
      k = 128 * PERM[s] + p.
    """
    k = np.arange(NPERSEG, dtype=np.float64)
    win = (0.5 - 0.5 * np.cos(2.0 * np.pi * k / NPERSEG)) * 2.0
    kb = np.arange(21, 21 + NBINS, dtype=np.float64)
    ang = 2.0 * np.pi * np.outer(k, kb) / NPERSEG
    nb_pad = sum(CHUNKS)
    C = np.zeros((NPERSEG, nb_pad), np.float32)
    S = np.zeros((NPERSEG, nb_pad), np.float32)
    C[:, :NBINS] = win[:, None] * np.cos(ang)
    S[:, :NBINS] = win[:, None] * np.sin(ang)
    out = {}
    for trig, M in (("c", C), ("s", S)):
        # [4096, 480] -> [p, j, n] -> permute j into slots
        M3 = M.reshape(32, 128, nb_pad).transpose(1, 0, 2)[:, PERM, :]
        col0 = 0
        for ci, rows in enumerate(CHUNKS):
            out[f"w{trig}{ci}"] = np.ascontiguousarray(
                M3[:, :, col0:col0 + rows]).astype(FP8)
            col0 += rows
    return out


_CACHE: dict = {}


def _get_prog():
    if "nc" not in _CACHE:
        _CACHE["nc"] = _build_nc()
    return _CACHE["nc"]


def _get_w():
    if "w" not in _CACHE:
        _CACHE["w"] = _build_w()
    return _CACHE["w"]


def _xs_layout(x: np.ndarray, core: int) -> np.ndarray:
    """[128 part, 16 slot, 512 grp] fp8 view of Welch row ROW0+core:
    xs[p, j, g] = R[2048 g + 128 j + p] where R is the row's 1048576
    samples (R[1024 b + m] = x[b, 1024 (ROW0+core) + m])."""
    c0 = (ROW0 + core) * 1024
    R = np.ascontiguousarray(x[:, c0:c0 + 1024]).reshape(-1).astype(FP8)
    return np.ascontiguousarray(R.reshape(512, 16, 128).transpose(2, 1, 0))


def kernel(pred: np.ndarray, target: np.ndarray, _trace: bool = False):
    nc = _get_prog()
    w = _get_w()
    pred = np.asarray(pred)
    target = np.asarray(target)
    in_maps = []
    for i in range(N_CORES):
        in_maps.append({
            "xst": _xs_layout(target, i),
            "xsp": _xs_layout(pred, i),
            **w,
        })
    res = run_bass_kernel_spmd(nc, in_maps, list(range(N_CORES)), trace=_trace)
    total = float(sum(float(res.results[i]["out"][0, 0])
                      for i in range(N_CORES)))
    out = np.array(total, dtype=np.float32)
    if _trace:
        return out, res
    return out
